# revision 14
# baseline (speedup 1.0000x reference)
"""BinaryConv2d (3x3, stride 1, pad 1) on 8 Trainium2 NeuronCores.

Data-parallel over batch: 32 images -> 4 per core, weights replicated.

Host prep: the binarized weight sign(w) (exactly +-1) goes to fp16 lhsT
layout [c, tap, k]; alpha is applied per output channel in fp32 during the
PSUM->SBUF eviction, so results are exact up to the fp16 input rounding.

Per-core kernel: images are processed in pairs. The pair's 2x64 input
channels fill the 128 SBUF partitions, each holding a zero-padded 114x114
fp16 image plane (fp32 DMA land + ScalarE cast). The 3x3 conv is 9
PSUM-accumulated matmuls per 4-row output chunk: lhsT = [c, k] tap weights,
rhs = the padded plane shifted by the tap offset (pure AP arithmetic).
Four matmul streams run concurrently on the four 64x64 PE array quadrants:
(image A, image B) x (chunk c, chunk c+1).
"""

import numpy as np

import concourse.bass as bass
import concourse.tile as tile
from concourse import bacc, mybir
from concourse.bass_utils import run_bass_kernel_spmd

N_CORES = 8
N_PER_CORE = 4  # images per core (batch 32 / 8 cores)
C = 64          # input channels
K = 64          # output channels
H = W = 112
HP, WP = H + 2, W + 2   # zero-padded plane
R = 4                   # output rows per PSUM half-chunk (R*W = 448 <= 512)
NSUPER = H // (2 * R)   # 14 superchunks (8 rows each) per image pair
SGROUP = 7              # superchunks per staged output DMA group
NBAND = 4               # input cast bands per pair (28 rows each)
BROWS = H // NBAND
F16 = mybir.dt.float16
F32 = mybir.dt.float32


def _build_nc(dyn_rep=False):
    """Build the per-core program. dyn_rep=True adds a "rep" [1,1] int32
    input and wraps the body in a hardware For_i loop with that runtime trip
    count (timing only; the computation is idempotent)."""
    nc = bacc.Bacc(
        "TRN2", target_bir_lowering=False, debug=False, num_devices=N_CORES
    )
    x_d = nc.dram_tensor("x", [N_PER_CORE, C, H, W], F32, kind="ExternalInput")
    wt_d = nc.dram_tensor("wt", [128, 9 * K], F16, kind="ExternalInput")
    al_d = nc.dram_tensor("al", [128, 1], F32, kind="ExternalInput")
    if dyn_rep:
        rep_d = nc.dram_tensor("rep", [1, 1], mybir.dt.int32, kind="ExternalInput")
    out_d = nc.dram_tensor("out", [N_PER_CORE, K, H, W], F32, kind="ExternalOutput")

    from contextlib import ExitStack, nullcontext

    with tile.TileContext(nc) as tc:
        rep_ctx = nullcontext()
        if dyn_rep:
            with tc.tile_pool(name="reppool", bufs=1) as reppool:
                rep_sb = reppool.tile([1, 1], mybir.dt.int32)
                nc.sync.dma_start(out=rep_sb[:], in_=rep_d[:])
                rv = nc.values_load(rep_sb[0:1, 0:1])
            rep_ctx = tc.For_i(
                0, rv, 1,
                hint_engines=(mybir.EngineType.PE, mybir.EngineType.SP,
                              mybir.EngineType.DVE, mybir.EngineType.Activation),
            )
        with (
            tc.tile_pool(name="wpool", bufs=1) as wpool,
            tc.tile_pool(name="rawpool", bufs=3) as rawpool,
            tc.tile_pool(name="xpool", bufs=2) as xpool,
            tc.tile_pool(name="opool", bufs=6) as opool,
            tc.tile_pool(name="pspool", bufs=8, space="PSUM") as pspool,
            rep_ctx,
        ):
            w_sb = wpool.tile([128, 9 * K], F16)
            nc.sync.dma_start(out=w_sb[:], in_=wt_d[:])
            al_sb = wpool.tile([128, 1], F32)
            nc.sync.dma_start(out=al_sb[:], in_=al_d[:])

            for pair in range(N_PER_CORE // 2):
                xpad = xpool.tile([128, HP * WP], F16)
                v = xpad.rearrange("p (h w) -> p h w", h=HP)
                # zero the padding border
                nc.vector.memset(v[:, 0, :], 0.0)
                nc.vector.memset(v[:, HP - 1, :], 0.0)
                nc.vector.memset(v[:, 1 : HP - 1, 0], 0.0)
                nc.vector.memset(v[:, 1 : HP - 1, WP - 1], 0.0)
                # land fp32 bands, cast+scatter into the fp16 padded plane
                for b in range(NBAND):
                    r0 = b * BROWS
                    xraw = rawpool.tile([128, BROWS * W], F32)
                    nc.sync.dma_start(
                        out=xraw[:],
                        in_=x_d[2 * pair : 2 * pair + 2, :, r0 : r0 + BROWS, :]
                        .rearrange("n c h w -> (n c) (h w)"),
                    )
                    nc.scalar.copy(
                        v[:, 1 + r0 : 1 + r0 + BROWS, 1 : W + 1],
                        xraw.rearrange("p (h w) -> p h w", h=BROWS),
                    )

                # partition p = (chunk_half, out_ch); rows interleave as
                # row = s*8 + chunk_half*4 + r
                outv = [
                    out_d[2 * pair + img].rearrange(
                        "c (ss hh r) w -> ss hh c (r w)", ss=NSUPER, hh=2, r=R
                    )
                    for img in (0, 1)
                ]
                for s in range(NSUPER):
                    y0 = s * 2 * R
                    psa = pspool.tile([128, R * W], F32, name="psa", tag="ps")
                    psb = pspool.tile([128, R * W], F32, name="psb", tag="ps")
                    for t in range(9):
                        dy, dx = divmod(t, 3)
                        for img in (0, 1):       # image within pair
                            p0 = img * 64
                            ps = (psa, psb)[img]
                            for ch in (0, 1):    # chunk half (4 rows each)
                                yy = y0 + ch * R + dy
                                nc.tensor.matmul(
                                    ps[ch * 64 : ch * 64 + 64, :],
                                    w_sb[p0 : p0 + 64, t * K : (t + 1) * K],
                                    v[p0 : p0 + 64, yy : yy + R, dx : dx + W],
                                    start=(t == 0),
                                    stop=(t == 8),
                                    skip_group_check=True,
                                )
                    for img in (0, 1):
                        ob = opool.tile([128, R * W], F32, name="ob", tag="ob")
                        nc.vector.tensor_scalar_mul(
                            ob[:], (psa, psb)[img][:], al_sb[:]
                        )
                        nc.sync.dma_start(out=outv[img][s], in_=ob[:])
    nc.compile()
    return nc


_NC_CACHE = None


def _get_nc():
    global _NC_CACHE
    if _NC_CACHE is None:
        _NC_CACHE = _build_nc()
    return _NC_CACHE


def _prep_weight(weight):
    weight = np.asarray(weight, dtype=np.float32)
    sgn = np.where(weight >= 0, np.float16(1.0), np.float16(-1.0)).astype(np.float16)
    arr = sgn.reshape(K, C, 9).transpose(1, 2, 0).reshape(C, 9 * K)  # [c, t*K + k]
    return np.ascontiguousarray(np.concatenate([arr, arr], axis=0))  # [128, 9K]


def _prep_alpha(alpha):
    a = np.asarray(alpha, dtype=np.float32).reshape(K, 1)
    return np.ascontiguousarray(np.concatenate([a, a], axis=0))  # [128, 1]


def run_sharded(inputs, trace=False, **kw):
    x = np.ascontiguousarray(np.asarray(inputs["input"], dtype=np.float32))
    wt = _prep_weight(inputs["weight"])
    al = _prep_alpha(inputs["alpha"])
    nc = _get_nc()
    in_maps = [
        {"x": x[i * N_PER_CORE : (i + 1) * N_PER_CORE], "wt": wt, "al": al}
        for i in range(N_CORES)
    ]
    res = run_bass_kernel_spmd(nc, in_maps, list(range(N_CORES)), trace=trace, **kw)
    out = np.concatenate(
        [res.results[i]["out"] for i in range(N_CORES)], axis=0
    )
    return out, res


def kernel(**inputs) -> np.ndarray:
    out, _ = run_sharded(inputs)
    return out


def _timed_runner(nc, inputs, extra=None):
    """Build a jitted 8-core runner for `nc` and device-resident args."""
    import jax
    from jax.experimental.shard_map import shard_map
    from jax.sharding import Mesh, NamedSharding, PartitionSpec

    from concourse import bass2jax

    bass2jax.install_neuronx_cc_hook()
    x = np.ascontiguousarray(np.asarray(inputs["input"], dtype=np.float32))
    wt = _prep_weight(inputs["weight"])
    al = _prep_alpha(inputs["alpha"])

    partition_name = nc.partition_id_tensor.name if nc.partition_id_tensor else None
    in_names, out_names, out_avals, zero_outs = [], [], [], []
    for alloc in nc.m.functions[0].allocations:
        if not isinstance(alloc, mybir.MemoryLocationSet):
            continue
        name = alloc.memorylocations[0].name
        if alloc.kind == "ExternalInput":
            if name != partition_name:
                in_names.append(name)
        elif alloc.kind == "ExternalOutput":
            shape = tuple(alloc.tensor_shape)
            dtype = mybir.dt.np(alloc.dtype)
            out_names.append(name)
            out_avals.append(jax.core.ShapedArray(shape, dtype))
            zero_outs.append(np.zeros(shape, dtype))
    n_params = len(in_names)

    def _body(*args):
        operands = list(args)
        if partition_name is not None:
            operands.append(bass2jax.partition_id_tensor())
        outs = bass2jax._bass_exec_p.bind(
            *operands,
            out_avals=tuple(out_avals),
            in_names=tuple(
                in_names + out_names + ([partition_name] if partition_name else [])
            ),
            out_names=tuple(out_names),
            lowering_input_output_aliases=(),
            sim_require_finite=True,
            sim_require_nnan=True,
            nc=nc,
        )
        return tuple(outs)

    devices = jax.devices()[:N_CORES]
    mesh = Mesh(np.asarray(devices), ("core",))
    spec = PartitionSpec("core")
    nshard = NamedSharding(mesh, spec)
    fn = jax.jit(
        shard_map(
            _body,
            mesh=mesh,
            in_specs=(spec,) * (n_params + len(out_names)),
            out_specs=(spec,) * len(out_names),
            check_rep=False,
        ),
        keep_unused=True,
    )
    per_core = {
        "x": [x[i * N_PER_CORE : (i + 1) * N_PER_CORE] for i in range(N_CORES)],
        "wt": [wt] * N_CORES,
        "al": [al] * N_CORES,
    }
    for name, arr in (extra or {}).items():
        per_core[name] = [arr] * N_CORES
    args = [np.concatenate(per_core[name], axis=0) for name in in_names] + [
        np.zeros((N_CORES * z.shape[0], *z.shape[1:]), z.dtype) for z in zero_outs
    ]
    dev_args = [jax.device_put(a, nshard) for a in args]
    idx = {name: i for i, name in enumerate(in_names)}
    return fn, dev_args, idx, nshard


def time_kernel(inputs, rep_big=257, pairs=6):
    """Isolate on-device kernel time with ONE executable whose For_i trip
    count is a runtime input: wall(rep_big) - wall(1), / (rep_big - 1).
    Alternates the two trip counts to cancel slow drift."""
    import time

    import jax

    nc = _build_nc(dyn_rep=True)
    fn, dev_args, idx, nshard = _timed_runner(
        nc, inputs, extra={"rep": np.array([[1]], np.int32)}
    )
    ri = idx["rep"]

    def arg_set(k):
        a = list(dev_args)
        a[ri] = jax.device_put(
            np.concatenate([np.array([[k]], np.int32)] * N_CORES, axis=0), nshard
        )
        return a

    a1, ab = arg_set(1), arg_set(rep_big)
    for a in (a1, ab):  # compile + warm both trip counts
        jax.block_until_ready(fn(*a))

    t1s, tbs = [], []
    for _ in range(pairs):
        t0 = time.perf_counter()
        jax.block_until_ready(fn(*a1))
        t1s.append(time.perf_counter() - t0)
        t0 = time.perf_counter()
        jax.block_until_ready(fn(*ab))
        tbs.append(time.perf_counter() - t0)
    t1, tb = min(t1s), min(tbs)
    per_exec = (tb - t1) / (rep_big - 1)
    return per_exec * 1e9, {"t1": t1s, "tbig": tbs, "rep_big": rep_big}


# revision 15
# speedup vs baseline: 3.3231x; 3.3231x over previous
"""BinaryConv2d (3x3, stride 1, pad 1) on 8 Trainium2 NeuronCores.

Data-parallel over batch: 32 images -> 4 per core, weights replicated.

Host prep: the binarized weight sign(w) (exactly +-1) goes to fp16 lhsT
layout [c, tap, k]; alpha is applied per output channel in fp32 during the
PSUM->SBUF eviction, so results are exact up to the fp16 input rounding.

Per-core kernel: images are processed in pairs. The pair's 2x64 input
channels fill the 128 SBUF partitions, each holding a zero-padded 114x114
fp16 image plane (fp32 DMA land + ScalarE cast). The 3x3 conv is 9
PSUM-accumulated matmuls per 4-row output chunk: lhsT = [c, k] tap weights,
rhs = the padded plane shifted by the tap offset (pure AP arithmetic).
Four matmul streams run concurrently on the four 64x64 PE array quadrants:
(image A, image B) x (chunk c, chunk c+1).
"""

import numpy as np

import concourse.bass as bass
import concourse.tile as tile
from concourse import bacc, mybir
from concourse.bass_utils import run_bass_kernel_spmd

N_CORES = 8
N_PER_CORE = 4  # images per core (batch 32 / 8 cores)
C = 64          # input channels
K = 64          # output channels
H = W = 112
HP, WP = H + 2, W + 2   # zero-padded plane
R = 4                   # output rows per PSUM half-chunk (R*W = 448 <= 512)
NSUPER = H // (2 * R)   # 14 superchunks (8 rows each) per image pair
SGROUP = 7              # superchunks per staged output DMA group
NBAND = 2               # input cast bands per pair (56 rows each)
BROWS = H // NBAND
F16 = mybir.dt.float16
F32 = mybir.dt.float32


def _build_nc(dyn_rep=False):
    """Build the per-core program. dyn_rep=True adds a "rep" [1,1] int32
    input and wraps the body in a hardware For_i loop with that runtime trip
    count (timing only; the computation is idempotent)."""
    nc = bacc.Bacc(
        "TRN2", target_bir_lowering=False, debug=False, num_devices=N_CORES
    )
    x_d = nc.dram_tensor("x", [N_PER_CORE, C, H, W], F32, kind="ExternalInput")
    wt_d = nc.dram_tensor("wt", [128, 9 * K], F16, kind="ExternalInput")
    al_d = nc.dram_tensor("al", [128, 1], F32, kind="ExternalInput")
    if dyn_rep:
        rep_d = nc.dram_tensor("rep", [1, 1], mybir.dt.int32, kind="ExternalInput")
    out_d = nc.dram_tensor("out", [N_PER_CORE, K, H, W], F32, kind="ExternalOutput")

    from contextlib import ExitStack, nullcontext

    with tile.TileContext(nc) as tc:
        rep_ctx = nullcontext()
        if dyn_rep:
            with tc.tile_pool(name="reppool", bufs=1) as reppool:
                rep_sb = reppool.tile([1, 1], mybir.dt.int32)
                nc.sync.dma_start(out=rep_sb[:], in_=rep_d[:])
                rv = nc.values_load(rep_sb[0:1, 0:1])
            rep_ctx = tc.For_i(
                0, rv, 1,
                hint_engines=(mybir.EngineType.PE, mybir.EngineType.SP,
                              mybir.EngineType.DVE, mybir.EngineType.Activation),
            )
        with (
            tc.tile_pool(name="wpool", bufs=1) as wpool,
            tc.tile_pool(name="rawpool", bufs=2) as rawpool,
            tc.tile_pool(name="xpool", bufs=2) as xpool,
            tc.tile_pool(name="opool", bufs=2) as opool,
            tc.tile_pool(name="pspool", bufs=8, space="PSUM") as pspool,
            rep_ctx,
        ):
            w_sb = wpool.tile([128, 9 * K], F16)
            nc.sync.dma_start(out=w_sb[:], in_=wt_d[:])
            al_sb = wpool.tile([128, 1], F32)
            nc.sync.dma_start(out=al_sb[:], in_=al_d[:])

            for pair in range(N_PER_CORE // 2):
                xpad = xpool.tile([128, HP * WP], F16)
                v = xpad.rearrange("p (h w) -> p h w", h=HP)
                # zero the padding border
                nc.vector.memset(v[:, 0, :], 0.0)
                nc.vector.memset(v[:, HP - 1, :], 0.0)
                nc.vector.memset(v[:, 1 : HP - 1, 0], 0.0)
                nc.vector.memset(v[:, 1 : HP - 1, WP - 1], 0.0)
                # land fp32 bands, cast+scatter into the fp16 padded plane
                for b in range(NBAND):
                    r0 = b * BROWS
                    xraw = rawpool.tile([128, BROWS * W], F32)
                    nc.sync.dma_start(
                        out=xraw[:],
                        in_=x_d[2 * pair : 2 * pair + 2, :, r0 : r0 + BROWS, :]
                        .rearrange("n c h w -> (n c) (h w)"),
                    )
                    nc.scalar.copy(
                        v[:, 1 + r0 : 1 + r0 + BROWS, 1 : W + 1],
                        xraw.rearrange("p (h w) -> p h w", h=BROWS),
                    )

                for g in range(NSUPER // SGROUP):
                    ost = [
                        opool.tile([128, SGROUP * R * W], F32, name=f"ost{i}", tag=f"ost{i}")
                        for i in range(2)
                    ]
                    for s in range(SGROUP):
                        y0 = (g * SGROUP + s) * 2 * R
                        psa = pspool.tile([128, R * W], F32, name="psa", tag="ps")
                        psb = pspool.tile([128, R * W], F32, name="psb", tag="ps")
                        for t in range(9):
                            dy, dx = divmod(t, 3)
                            for img in (0, 1):       # image within pair
                                p0 = img * 64
                                ps = (psa, psb)[img]
                                for ch in (0, 1):    # chunk half (4 rows each)
                                    yy = y0 + ch * R + dy
                                    nc.tensor.matmul(
                                        ps[ch * 64 : ch * 64 + 64, :],
                                        w_sb[p0 : p0 + 64, t * K : (t + 1) * K],
                                        v[p0 : p0 + 64, yy : yy + R, dx : dx + W],
                                        start=(t == 0),
                                        stop=(t == 8),
                                        skip_group_check=True,
                                    )
                        for img in (0, 1):
                            nc.vector.tensor_scalar_mul(
                                ost[img][:, s * R * W : (s + 1) * R * W],
                                (psa, psb)[img][:],
                                al_sb[:],
                            )
                    for img in (0, 1):
                        # partition p = (chunk_half, out_ch); rows interleave as
                        # row = g*56 + s*8 + chunk_half*4 + r
                        dstv = out_d[2 * pair + img].rearrange(
                            "c (gg s hh r) w -> gg hh c s (r w)",
                            gg=NSUPER // SGROUP, s=SGROUP, hh=2, r=R,
                        )[g]
                        srcv = ost[img].rearrange("p (s rw) -> p s rw", s=SGROUP)
                        for hh in (0, 1):
                            nc.sync.dma_start(
                                out=dstv[hh],
                                in_=srcv[hh * 64 : (hh + 1) * 64],
                            )
    nc.compile()
    return nc


_NC_CACHE = None


def _get_nc():
    global _NC_CACHE
    if _NC_CACHE is None:
        _NC_CACHE = _build_nc()
    return _NC_CACHE


def _prep_weight(weight):
    weight = np.asarray(weight, dtype=np.float32)
    sgn = np.where(weight >= 0, np.float16(1.0), np.float16(-1.0)).astype(np.float16)
    arr = sgn.reshape(K, C, 9).transpose(1, 2, 0).reshape(C, 9 * K)  # [c, t*K + k]
    return np.ascontiguousarray(np.concatenate([arr, arr], axis=0))  # [128, 9K]


def _prep_alpha(alpha):
    a = np.asarray(alpha, dtype=np.float32).reshape(K, 1)
    return np.ascontiguousarray(np.concatenate([a, a], axis=0))  # [128, 1]


def run_sharded(inputs, trace=False, **kw):
    x = np.ascontiguousarray(np.asarray(inputs["input"], dtype=np.float32))
    wt = _prep_weight(inputs["weight"])
    al = _prep_alpha(inputs["alpha"])
    nc = _get_nc()
    in_maps = [
        {"x": x[i * N_PER_CORE : (i + 1) * N_PER_CORE], "wt": wt, "al": al}
        for i in range(N_CORES)
    ]
    res = run_bass_kernel_spmd(nc, in_maps, list(range(N_CORES)), trace=trace, **kw)
    out = np.concatenate(
        [res.results[i]["out"] for i in range(N_CORES)], axis=0
    )
    return out, res


def kernel(**inputs) -> np.ndarray:
    out, _ = run_sharded(inputs)
    return out


def _timed_runner(nc, inputs, extra=None):
    """Build a jitted 8-core runner for `nc` and device-resident args."""
    import jax
    from jax.experimental.shard_map import shard_map
    from jax.sharding import Mesh, NamedSharding, PartitionSpec

    from concourse import bass2jax

    bass2jax.install_neuronx_cc_hook()
    x = np.ascontiguousarray(np.asarray(inputs["input"], dtype=np.float32))
    wt = _prep_weight(inputs["weight"])
    al = _prep_alpha(inputs["alpha"])

    partition_name = nc.partition_id_tensor.name if nc.partition_id_tensor else None
    in_names, out_names, out_avals, zero_outs = [], [], [], []
    for alloc in nc.m.functions[0].allocations:
        if not isinstance(alloc, mybir.MemoryLocationSet):
            continue
        name = alloc.memorylocations[0].name
        if alloc.kind == "ExternalInput":
            if name != partition_name:
                in_names.append(name)
        elif alloc.kind == "ExternalOutput":
            shape = tuple(alloc.tensor_shape)
            dtype = mybir.dt.np(alloc.dtype)
            out_names.append(name)
            out_avals.append(jax.core.ShapedArray(shape, dtype))
            zero_outs.append(np.zeros(shape, dtype))
    n_params = len(in_names)

    def _body(*args):
        operands = list(args)
        if partition_name is not None:
            operands.append(bass2jax.partition_id_tensor())
        outs = bass2jax._bass_exec_p.bind(
            *operands,
            out_avals=tuple(out_avals),
            in_names=tuple(
                in_names + out_names + ([partition_name] if partition_name else [])
            ),
            out_names=tuple(out_names),
            lowering_input_output_aliases=(),
            sim_require_finite=True,
            sim_require_nnan=True,
            nc=nc,
        )
        return tuple(outs)

    devices = jax.devices()[:N_CORES]
    mesh = Mesh(np.asarray(devices), ("core",))
    spec = PartitionSpec("core")
    nshard = NamedSharding(mesh, spec)
    fn = jax.jit(
        shard_map(
            _body,
            mesh=mesh,
            in_specs=(spec,) * (n_params + len(out_names)),
            out_specs=(spec,) * len(out_names),
            check_rep=False,
        ),
        keep_unused=True,
    )
    per_core = {
        "x": [x[i * N_PER_CORE : (i + 1) * N_PER_CORE] for i in range(N_CORES)],
        "wt": [wt] * N_CORES,
        "al": [al] * N_CORES,
    }
    for name, arr in (extra or {}).items():
        per_core[name] = [arr] * N_CORES
    args = [np.concatenate(per_core[name], axis=0) for name in in_names] + [
        np.zeros((N_CORES * z.shape[0], *z.shape[1:]), z.dtype) for z in zero_outs
    ]
    dev_args = [jax.device_put(a, nshard) for a in args]
    idx = {name: i for i, name in enumerate(in_names)}
    return fn, dev_args, idx, nshard


def time_kernel(inputs, rep_big=257, pairs=6):
    """Isolate on-device kernel time with ONE executable whose For_i trip
    count is a runtime input: wall(rep_big) - wall(1), / (rep_big - 1).
    Alternates the two trip counts to cancel slow drift."""
    import time

    import jax

    nc = _build_nc(dyn_rep=True)
    fn, dev_args, idx, nshard = _timed_runner(
        nc, inputs, extra={"rep": np.array([[1]], np.int32)}
    )
    ri = idx["rep"]

    def arg_set(k):
        a = list(dev_args)
        a[ri] = jax.device_put(
            np.concatenate([np.array([[k]], np.int32)] * N_CORES, axis=0), nshard
        )
        return a

    a1, ab = arg_set(1), arg_set(rep_big)
    for a in (a1, ab):  # compile + warm both trip counts
        jax.block_until_ready(fn(*a))

    t1s, tbs = [], []
    for _ in range(pairs):
        t0 = time.perf_counter()
        jax.block_until_ready(fn(*a1))
        t1s.append(time.perf_counter() - t0)
        t0 = time.perf_counter()
        jax.block_until_ready(fn(*ab))
        tbs.append(time.perf_counter() - t0)
    t1, tb = min(t1s), min(tbs)
    per_exec = (tb - t1) / (rep_big - 1)
    return per_exec * 1e9, {"t1": t1s, "tbig": tbs, "rep_big": rep_big}
